# revision 1
# baseline (speedup 1.0000x reference)
"""CAPAttentionModule Trainium2 kernel.

Data-parallel over batch: 8 images -> 8 NeuronCores, one image per core.
Per core (x: [512, 9216] = [C, H*W], H=W=96):
  k1 = relu(Wkp x + b)              [128, HW]   (1x1 conv, BN folded)
  k2 = relu(dw3x3(k1) + b)          [128, HW]   (depthwise via diagonal matmuls)
  v1 = relu(Wvp x + b)              [256, HW]
  v2 = relu(dw3x3(v1) + b)          [256, HW]
  key = psp([k1;k2])   [256, 110],  value = psp([v1;v2])  [512, 110]
  q  = relu(Wq x + b)               [256, HW]
  sim = softmax_s(q^T key / 16)     [HW, 110]   (no max-subtract; |sim|<4)
  out = x + value @ sim^T           [512, HW]

All matmuls use float32r (full-rate fp32 on the PE at N>=256).
Depthwise 3x3 runs as 9 shifted diagonal matmuls accumulating in PSUM;
SAME-padding comes from a zero column pad (width 98 layout) plus
row-restricted APs at the image top/bottom (has_written overwrite
semantics make ragged accumulation exact).
PSP pooling: one 5D strided reduce to a 24x24 sum grid per map, then
small batched reduces for the 1/3/6/8 grids; normalization (and the
1/sqrt(256) sim scale) is folded into per-s scale tiles.
"""

import numpy as np

P = 128
HH = 96
WP = 98          # padded width/height (zero border ring)
HW = 9216
HWP = WP * WP    # 9604: [98, 98] with zero border, data at [1:97, 1:97]
RB = 24          # row blocks of 4 rows
RBN = 4 * HH     # 384
NCH = 18         # phase-B column chunks
NCW = 512
DWG = 6          # dw row-blocks per psum group
S = 110


def _f32r(ap):
    from concourse import mybir
    return ap.bitcast(mybir.dt.float32r)




def bass_ap_pool_view(ap_rows):
    """[p, >=4*WP] AP at the start of 4 data rows (stride WP) ->
    [p, wq, h, ws] view for a 4x4 pooling reduce over (h, ws)."""
    v = ap_rows[:, 0:4 * WP].rearrange("p (h w) -> p h w", w=WP)
    v = v[:, :, 0:HH]
    return v.rearrange("p h (wq ws) -> p wq h ws", ws=4)

def build_bass():
    import concourse.bacc as bacc
    import concourse.tile as tile
    from concourse import mybir
    from contextlib import ExitStack

    f32 = mybir.dt.float32
    f32r = mybir.dt.float32r
    bf16 = mybir.dt.bfloat16
    AF = mybir.ActivationFunctionType
    AX = mybir.AxisListType

    nc = bacc.Bacc("TRN2", target_bir_lowering=False, debug=False,
                   enable_asserts=False, num_devices=8)

    x_d = nc.dram_tensor("x", [512, HW], f32r, kind="ExternalInput").ap()
    xb_d = nc.dram_tensor("xb", [512, HW], bf16, kind="ExternalInput").ap()
    wq_d = nc.dram_tensor("wq", [512, 256], bf16, kind="ExternalInput").ap()
    wkp_d = nc.dram_tensor("wkp", [512, 128], bf16, kind="ExternalInput").ap()
    wvp_d = nc.dram_tensor("wvp", [512, 256], bf16, kind="ExternalInput").ap()
    diag_d = nc.dram_tensor("diag", [3, 9, 128, 128], bf16, kind="ExternalInput").ap()
    id_d = nc.dram_tensor("ident", [128, 128], bf16, kind="ExternalInput").ap()
    scl_d = nc.dram_tensor("scl", [2, 128, S], f32, kind="ExternalInput").ap()
    bias_d = nc.dram_tensor("bias", [128, 8], f32, kind="ExternalInput").ap()
    y_d = nc.dram_tensor("y", [512, HW], f32, kind="ExternalOutput").ap()

    x_r = x_d.rearrange("(t p) n -> p t n", p=P)      # [128, 4, 9216]
    xb_r = xb_d.rearrange("(t p) n -> p t n", p=P)
    y_r = y_d.rearrange("(t p) n -> p t n", p=P)

    with tile.TileContext(nc) as tc:
        with ExitStack() as top:
            cpool = top.enter_context(tc.tile_pool(name="consts", bufs=1))
            kpool = top.enter_context(tc.tile_pool(name="keep", bufs=1))

            c_wq = cpool.tile([P, 4 * 256], bf16)
            nc.sync.dma_start(c_wq[:].rearrange("p (t m) -> p t m", t=4),
                              wq_d.rearrange("(t p) m -> p t m", p=P))
            c_wkp = cpool.tile([P, 4 * 128], bf16)
            nc.sync.dma_start(c_wkp[:].rearrange("p (t m) -> p t m", t=4),
                              wkp_d.rearrange("(t p) m -> p t m", p=P))
            c_wvp = cpool.tile([P, 4 * 256], bf16)
            nc.sync.dma_start(c_wvp[:].rearrange("p (t m) -> p t m", t=4),
                              wvp_d.rearrange("(t p) m -> p t m", p=P))
            c_dg = cpool.tile([P, 27 * 128], bf16)
            nc.sync.dma_start(c_dg[:].rearrange("p (ct m) -> p ct m", ct=27),
                              diag_d.rearrange("c t p m -> p (c t) m"))
            c_id = cpool.tile([P, 128], bf16)
            nc.sync.dma_start(c_id[:], id_d)
            c_scl = cpool.tile([P, 2 * S], f32)
            nc.sync.dma_start(c_scl[:].rearrange("p (s m) -> p s m", s=2),
                              scl_d.rearrange("s p m -> p s m"))
            c_bias = cpool.tile([P, 8], f32)
            nc.sync.dma_start(c_bias[:], bias_d)

            keyn = kpool.tile([P, 2 * S], bf16)       # normalized key (incl /16)
            vT = kpool.tile([S, 512], bf16)           # value^T [s, c]

            # ---------------- Phase A: key/value branches ----------------
            with ExitStack() as actx:
                bigp = actx.enter_context(tc.tile_pool(name="bigA", bufs=1))
                xap = actx.enter_context(tc.tile_pool(name="xa", bufs=3))
                blkp = actx.enter_context(tc.tile_pool(name="blk", bufs=6))
                tmpp = actx.enter_context(tc.tile_pool(name="tmpA", bufs=1))

                k1p = bigp.tile([P, HWP], bf16)
                v1p = bigp.tile([P, 2 * HWP], bf16)
                p24 = bigp.tile([P, 6 * 576], f32)
                allp = bigp.tile([P, 6 * S], f32)
                valn = bigp.tile([P, 4 * S], bf16)

                # zero the pad border (rows 0/97, cols 0/97)
                for chv in (k1p[:, 0:HWP], v1p[:, 0:HWP], v1p[:, HWP:2 * HWP]):
                    c3 = chv.rearrange("p (h w) -> p h w", w=WP)
                    nc.gpsimd.memset(c3[:, 0:1, :], 0.0)
                    nc.gpsimd.memset(c3[:, 97:98, :], 0.0)
                    nc.gpsimd.memset(c3[:, 1:97, 0:1], 0.0)
                    nc.gpsimd.memset(c3[:, 1:97, 97:98], 0.0)

                # primary 1x1 convs, streamed by 4-row blocks (2 blocks/DMA),
                # with per-block pooling of k1/v1a/v1b interleaved on DVE
                with tc.tile_pool(name="psA", bufs=2, space="PSUM") as psA:
                    for rbb in range(RB // 2):
                        xt = xap.tile([P, 4 * 2 * RBN], bf16, name="xt")
                        nc.sync.dma_start(
                            xt[:].rearrange("p (t n) -> p t n", t=4),
                            xb_r[:, :, rbb * 2 * RBN:(rbb + 1) * 2 * RBN])
                        dsts = [
                            (k1p, 0, c_wkp, 128, 0, 0),
                            (v1p, 0, c_wvp, 256, 2, 2),
                            (v1p, 1, c_wvp, 256, 3, 3),
                        ]
                        for sub in range(2):
                            rb = rbb * 2 + sub
                            for di, (dst, half, wt, wm, bcol, slot) in enumerate(dsts):
                                ps = psA.tile([P, RBN], f32, name=f"pps{di}")
                                for cc in range(4):
                                    lo = cc * wm + (half * 128 if wm == 256 else 0)
                                    nc.tensor.matmul(
                                        ps[:], wt[:, lo:lo + 128],
                                        xt[:, cc * 2 * RBN + sub * RBN:
                                           cc * 2 * RBN + (sub + 1) * RBN],
                                        start=(cc == 0), stop=(cc == 3))
                                dv = dst[:, half * HWP:(half + 1) * HWP].rearrange(
                                    "p (h w) -> p h w", w=WP)
                                nc.scalar.activation(
                                    dv[:, 4 * rb + 1:4 * rb + 5, 1:97],
                                    ps[:].rearrange("p (h w) -> p h w", w=HH),
                                    AF.Relu, bias=c_bias[:, bcol:bcol + 1])
                                st = (4 * rb + 1) * WP + 1
                                pv = dst[:, half * HWP + st:half * HWP + st + 4 * WP]
                                pv = bass_ap_pool_view(pv)
                                nc.vector.reduce_sum(
                                    p24[:, slot * 576 + rb * 24:slot * 576 + (rb + 1) * 24],
                                    pv, axis=AX.XY)

                # small pools over a map range [m0, m1) -> allp columns
                def smallpools(m0, m1):
                    m = m1 - m0
                    allp_v = allp[:, m0 * S:m1 * S].rearrange(
                        "p (m s) -> p m s", s=S)
                    p24s = p24[:, m0 * 576:m1 * 576]
                    nc.vector.reduce_sum(
                        allp_v[:, :, 0:1],
                        p24s.rearrange("p (m s) -> p m s", s=576), axis=AX.X)
                    tmp = tmpp.tile([P, 1152], f32, name="tmp", tag="tmp")
                    nc.vector.reduce_sum(
                        tmp[:, 0:m * 72],
                        p24s.rearrange("p (mh wq ws) -> p mh wq ws", wq=3, ws=8),
                        axis=AX.X)
                    nc.vector.reduce_sum(
                        allp_v[:, :, 1:10],
                        tmp[:, 0:m * 72].rearrange(
                            "p (m hq hs wq) -> p m hq wq hs", m=m, hq=3, hs=8),
                        axis=AX.X)
                    tmp6 = tmpp.tile([P, 1152], f32, name="tmp6", tag="tmp")
                    nc.vector.reduce_sum(
                        tmp6[:, 0:m * 144],
                        p24s.rearrange("p (mh wq ws) -> p mh wq ws", wq=6, ws=4),
                        axis=AX.X)
                    nc.vector.reduce_sum(
                        allp_v[:, :, 10:46],
                        tmp6[:, 0:m * 144].rearrange(
                            "p (m hq hs wq) -> p m hq wq hs", m=m, hq=6, hs=4),
                        axis=AX.X)
                    tmp8 = tmpp.tile([P, 1152], f32, name="tmp8", tag="tmp")
                    nc.vector.reduce_sum(
                        tmp8[:, 0:m * 192],
                        p24s.rearrange("p (mh wq ws) -> p mh wq ws", wq=8, ws=3),
                        axis=AX.X)
                    nc.vector.reduce_sum(
                        allp_v[:, :, 46:110],
                        tmp8[:, 0:m * 192].rearrange(
                            "p (m hq hs wq) -> p m hq wq hs", m=m, hq=8, hs=3),
                        axis=AX.X)


                # depthwise 3x3 via diagonal matmuls + pooling of k2/v2;
                # value maps pooled/transposed as soon as each is complete
                def vt_build(j):
                    tp = psTp.tile([P, 128], bf16, name="tp", tag="tp")
                    nc.tensor.transpose(tp[0:S, :], valn[:, j * S:(j + 1) * S],
                                        c_id[:])
                    nc.scalar.copy(vT[:, j * 128:(j + 1) * 128], tp[0:S, :])

                def val_finish(m0, m1):
                    smallpools(m0, m1)
                    for mm in range(m0, m1):
                        j = mm - 2
                        nc.vector.tensor_mul(valn[:, j * S:(j + 1) * S],
                                             allp[:, mm * S:(mm + 1) * S],
                                             c_scl[:, S:2 * S])
                        vt_build(j)

                with tc.tile_pool(name="psD", bufs=1, space="PSUM") as psD, \
                        tc.tile_pool(name="psTa", bufs=2, space="PSUM") as psTp:
                    # maps 2,3 (v1a, v1b) complete after the primary loop
                    val_finish(2, 4)
                    chunks = [(k1p[:, 0:HWP], 0, 1, 1),
                              (v1p[:, 0:HWP], 1, 4, 4),
                              (v1p[:, HWP:2 * HWP], 2, 5, 5)]
                    for chv, ci, bcol, slot in chunks:
                        ch3 = chv.rearrange("p (h w) -> p h w", w=WP)
                        for g in range(RB // DWG):
                            pss = [psD.tile([P, RBN], f32, name=f"dw{j}")
                                   for j in range(DWG)]
                            for t in range(9):
                                dy, dx = t // 3, t % 3
                                dgap = c_dg[:, (ci * 9 + t) * 128:(ci * 9 + t + 1) * 128]
                                for j in range(DWG):
                                    r0 = (g * DWG + j) * 4
                                    rhs = ch3[:, r0 + dy:r0 + dy + 4, dx:dx + HH]
                                    nc.tensor.matmul(
                                        pss[j][:], dgap, rhs,
                                        start=(t == 0), stop=(t == 8))
                            for j in range(DWG):
                                rb = g * DWG + j
                                blk = blkp.tile([P, RBN], bf16, name="blk")
                                nc.scalar.activation(
                                    blk[:], pss[j][:], AF.Relu,
                                    bias=c_bias[:, bcol:bcol + 1])
                                bv = blk[:].rearrange(
                                    "p (h wq ws) -> p wq h ws", h=4, ws=4)
                                nc.vector.reduce_sum(
                                    p24[:, slot * 576 + rb * 24:slot * 576 + (rb + 1) * 24],
                                    bv, axis=AX.XY)
                        if ci == 0:
                            # key branch done: pool + normalize immediately so
                            # phase-B sim/softmax can overlap the value chunks
                            smallpools(0, 2)
                            for kq in range(2):
                                nc.vector.tensor_mul(
                                    keyn[:, kq * S:(kq + 1) * S],
                                    allp[:, kq * S:(kq + 1) * S], c_scl[:, 0:S])
                        elif ci == 1:
                            val_finish(4, 5)
                        else:
                            val_finish(5, 6)


            # ---------------- Phase B: query / attention / output ----------------
            with ExitStack() as bctx:
                xbp = bctx.enter_context(tc.tile_pool(name="xb", bufs=5))
                xqp = bctx.enter_context(tc.tile_pool(name="xq", bufs=5))
                qp = bctx.enter_context(tc.tile_pool(name="qsb", bufs=5))
                pp = bctx.enter_context(tc.tile_pool(name="pexp", bufs=8))
                sp = bctx.enter_context(tc.tile_pool(name="small", bufs=8))
                stp = bctx.enter_context(tc.tile_pool(name="simT", bufs=5))
                obp = bctx.enter_context(tc.tile_pool(name="outb", bufs=3))
                psQ = bctx.enter_context(tc.tile_pool(name="psQ", bufs=1, space="PSUM"))
                psS = bctx.enter_context(tc.tile_pool(name="psS", bufs=2, space="PSUM"))
                psT2 = bctx.enter_context(tc.tile_pool(name="psT2", bufs=2, space="PSUM"))
                psC = bctx.enter_context(tc.tile_pool(name="psC", bufs=2, space="PSUM"))

                for n in range(NCH):
                    xt = xbp.tile([P, 4 * NCW], f32r, name="xtb")
                    nc.sync.dma_start(
                        xt[:].rearrange("p (t n) -> p t n", t=4),
                        x_r[:, :, n * NCW:(n + 1) * NCW])
                    xtb = xqp.tile([P, 4 * NCW], bf16, name="xtq")
                    nc.sync.dma_start(
                        xtb[:].rearrange("p (t n) -> p t n", t=4),
                        xb_r[:, :, n * NCW:(n + 1) * NCW])
                    qsb = qp.tile([P, 2 * NCW], bf16, name="qsb")
                    for kq in range(2):
                        qps = psQ.tile([P, NCW], f32, name=f"q{kq}")
                        for cc in range(4):
                            lo = cc * 256 + kq * 128
                            nc.tensor.matmul(
                                qps[:], c_wq[:, lo:lo + 128],
                                xtb[:, cc * NCW:(cc + 1) * NCW],
                                start=(cc == 0), stop=(cc == 3))
                        nc.scalar.activation(qsb[:, kq * NCW:(kq + 1) * NCW],
                                             qps[:], AF.Relu,
                                             bias=c_bias[:, 6 + kq:7 + kq])
                    sT = stp.tile([S, NCW], bf16, name="sT")
                    for ns in range(4):
                        sps = psS.tile([P, S], f32, name="sim")
                        for kq in range(2):
                            nc.tensor.matmul(
                                sps[:],
                                qsb[:, kq * NCW + ns * 128:kq * NCW + (ns + 1) * 128],
                                keyn[:, kq * S:(kq + 1) * S],
                                start=(kq == 0), stop=(kq == 1))
                        pe = pp.tile([P, S], bf16, name="pe")
                        sums = sp.tile([P, 1], f32, name="sums")
                        nc.scalar.activation(pe[:], sps[:], AF.Exp)
                        nc.vector.reduce_sum(sums[:], pe[:], axis=AX.X)
                        rp = sp.tile([P, 1], f32, name="rp")
                        nc.vector.reciprocal(rp[:], sums[:])
                        pn = pp.tile([P, S], bf16, name="pn")
                        nc.vector.tensor_scalar_mul(pn[:], pe[:], rp[:])
                        tp2 = psT2.tile([P, 128], bf16, name="tp2")
                        nc.tensor.transpose(tp2[0:S, :], pn[:], c_id[:])
                        nc.scalar.copy(sT[:, ns * 128:(ns + 1) * 128], tp2[0:S, :])
                    outb = obp.tile([P, 4 * NCW], f32, name="outb")
                    for cv in range(4):
                        cps = psC.tile([P, NCW], f32, name="ctx")
                        nc.tensor.matmul(cps[:], vT[:, cv * 128:(cv + 1) * 128],
                                         sT[:], start=True, stop=True)
                        nc.vector.tensor_add(outb[:, cv * NCW:(cv + 1) * NCW],
                                             cps[:], xt.bitcast(f32)[:, cv * NCW:(cv + 1) * NCW])
                    # store on the ScalarE HWDGE ring: keeps a resid-delayed
                    # store from head-of-line blocking the sync-ring x loads
                    nc.scalar.dma_start(
                        y_r[:, :, n * NCW:(n + 1) * NCW],
                        outb[:].rearrange("p (t n) -> p t n", t=4))

    nc.compile()
    return nc


def prep_host_inputs(inputs):
    """Fold BN affine into weights, build diag/scale/bias aux tensors."""
    g = lambda a: np.ascontiguousarray(np.asarray(a, dtype=np.float32))
    wq = (g(inputs["q_g"])[:, None] * g(inputs["q_w"])[:, :, 0, 0]).T
    wkp = (g(inputs["kp_g"])[:, None] * g(inputs["kp_w"])[:, :, 0, 0]).T
    wvp = (g(inputs["vp_g"])[:, None] * g(inputs["vp_w"])[:, :, 0, 0]).T
    wkc = g(inputs["kc_g"])[:, None] * g(inputs["kc_w"])[:, 0].reshape(128, 9)
    wvc = g(inputs["vc_g"])[:, None] * g(inputs["vc_w"])[:, 0].reshape(256, 9)

    diag = np.zeros((3, 9, 128, 128), np.float32)
    for t in range(9):
        diag[0, t] = np.diag(wkc[:, t])
        diag[1, t] = np.diag(wvc[:128, t])
        diag[2, t] = np.diag(wvc[128:, t])

    scale110 = np.zeros(S, np.float32)
    scale110[0] = 1.0 / 9216
    scale110[1:10] = 1.0 / 1024
    scale110[10:46] = 1.0 / 256
    scale110[46:110] = 1.0 / 144
    scl = np.zeros((2, 128, S), np.float32)
    scl[0] = scale110 / 16.0
    scl[1] = scale110

    bias = np.zeros((128, 8), np.float32)
    bias[:, 0] = g(inputs["kp_b"])
    bias[:, 1] = g(inputs["kc_b"])
    bias[:, 2] = g(inputs["vp_b"])[:128]
    bias[:, 3] = g(inputs["vp_b"])[128:]
    bias[:, 4] = g(inputs["vc_b"])[:128]
    bias[:, 5] = g(inputs["vc_b"])[128:]
    bias[:, 6] = g(inputs["q_b"])[:128]
    bias[:, 7] = g(inputs["q_b"])[128:]

    import ml_dtypes
    return {
        "wq": np.ascontiguousarray(wq).astype(ml_dtypes.bfloat16),
        "wkp": np.ascontiguousarray(wkp).astype(ml_dtypes.bfloat16),
        "wvp": np.ascontiguousarray(wvp).astype(ml_dtypes.bfloat16),
        "diag": diag.astype(ml_dtypes.bfloat16),
        "ident": np.eye(128, dtype=ml_dtypes.bfloat16),
        "scl": scl,
        "bias": bias,
    }


def make_in_maps(inputs):
    host = prep_host_inputs(inputs)
    x = np.asarray(inputs["x"], dtype=np.float32)
    B = x.shape[0]
    in_maps = []
    import ml_dtypes
    for b in range(B):
        m = dict(host)
        m["x"] = np.ascontiguousarray(x[b].reshape(512, HW))
        m["xb"] = m["x"].astype(ml_dtypes.bfloat16)
        in_maps.append(m)
    return in_maps


_NC = None


def get_nc():
    global _NC
    if _NC is None:
        _NC = build_bass()
    return _NC


def kernel(**inputs):
    from concourse import bass_utils
    nc = get_nc()
    in_maps = make_in_maps(inputs)
    res = bass_utils.run_bass_kernel_spmd(
        nc, in_maps, core_ids=list(range(len(in_maps))), trace=False)
    outs = [r["y"].reshape(512, HH, HH) for r in res.results]
    return np.stack(outs, axis=0).astype(np.float32)



# revision 4
# speedup vs baseline: 1.0606x; 1.0606x over previous
"""CAPAttentionModule Trainium2 kernel (v2: fp8 DoubleRow + transposed sim).

Data-parallel over batch: 8 images -> 8 NeuronCores, one image per core.
Per core (x: [512, 9216] = [C, H*W], H=W=96):
  k1 = relu(Wkp x + b)              [128, HW]   fp8 DoubleRow conv (K=512)
  v1 = relu(Wvp x + b)              [256, HW]   fp8 DoubleRow conv
  q  = relu(Wq x + b)               [256, HW]   fp8 DoubleRow conv, kept in fp8
  k2/v2 = relu(dw3x3 + b)           diag matmuls: 2 fp8 DoubleRow tap-pairs
                                    ((2,3),(5,6): delta=96, 16B-aligned) + 5 singles
  key/value = psp pooling           [*, 110]    batched strip reduces on DVE
  simT = key^T q / 16               [110, HW]   ONE fp8 DoubleRow matmul per 512-chunk
  softmax over s (partition dim):   exp on ACT; sum via ones-matmul (broadcast to
                                    all partitions); reciprocal+scale on DVE
  out = x + value @ simT            residual added half on DVE, half via
                                    identity-matmul PSUM accumulation + ACT copy
HBM traffic: read x fp8 (4.7MB) + x bf16 (9.4MB), write y bf16 (9.4MB).
"""

import numpy as np

P = 128
HH = 96
WP = 98          # padded width (zero border ring)
HW = 9216
HWP = WP * WP    # 9604
HWP2 = HWP + 2   # +1 lead pad (for fp8 DR 2B-aligned starts) +1 tail
RB = 24          # row blocks of 4 rows
RBN = 4 * HH     # 384
NCH = 18         # phase-B column chunks
NCW = 512
S = 110


def build_bass():
    import concourse.bacc as bacc
    import concourse.tile as tile
    from concourse import mybir, bass
    from contextlib import ExitStack

    f32 = mybir.dt.float32
    f8 = mybir.dt.float8e4
    bf16 = mybir.dt.bfloat16
    AF = mybir.ActivationFunctionType
    AX = mybir.AxisListType
    ALU = mybir.AluOpType
    DR = mybir.MatmulPerfMode.DoubleRow

    nc = bacc.Bacc("TRN2", target_bir_lowering=False, debug=False,
                   enable_asserts=False, num_devices=8)

    x8_d = nc.dram_tensor("x8", [512, HW], f8, kind="ExternalInput").ap()
    xb_d = nc.dram_tensor("xb", [512, HW], bf16, kind="ExternalInput").ap()
    wq_d = nc.dram_tensor("wq", [512, 256], f8, kind="ExternalInput").ap()
    wkp_d = nc.dram_tensor("wkp", [512, 128], f8, kind="ExternalInput").ap()
    wvp_d = nc.dram_tensor("wvp", [512, 256], f8, kind="ExternalInput").ap()
    diag_d = nc.dram_tensor("diag", [3, 9, 128, 128], f8, kind="ExternalInput").ap()
    id_d = nc.dram_tensor("ident", [128, 128], bf16, kind="ExternalInput").ap()
    ones_d = nc.dram_tensor("ones", [128, 128], bf16, kind="ExternalInput").ap()
    scl_d = nc.dram_tensor("scl", [2, 128, S], f32, kind="ExternalInput").ap()
    bias_d = nc.dram_tensor("bias", [128, 8], f32, kind="ExternalInput").ap()
    y_d = nc.dram_tensor("y", [512, HW], bf16, kind="ExternalOutput").ap()

    x8_r = x8_d.rearrange("(t p) n -> p t n", p=P)
    xb_r = xb_d.rearrange("(t p) n -> p t n", p=P)
    y_r = y_d.rearrange("(t p) n -> p t n", p=P)

    with tile.TileContext(nc) as tc:
        with ExitStack() as top:
            cpool = top.enter_context(tc.tile_pool(name="consts", bufs=1))
            kpool = top.enter_context(tc.tile_pool(name="keep", bufs=1))

            c_wq = cpool.tile([P, 4 * 256], f8)
            nc.sync.dma_start(c_wq[:].rearrange("p (t m) -> p t m", t=4),
                              wq_d.rearrange("(t p) m -> p t m", p=P))
            c_wkp = cpool.tile([P, 4 * 128], f8)
            nc.sync.dma_start(c_wkp[:].rearrange("p (t m) -> p t m", t=4),
                              wkp_d.rearrange("(t p) m -> p t m", p=P))
            c_wvp = cpool.tile([P, 4 * 256], f8)
            nc.sync.dma_start(c_wvp[:].rearrange("p (t m) -> p t m", t=4),
                              wvp_d.rearrange("(t p) m -> p t m", p=P))
            c_dg = cpool.tile([P, 27 * 128], f8)
            nc.sync.dma_start(c_dg[:].rearrange("p (ct m) -> p ct m", ct=27),
                              diag_d.rearrange("c t p m -> p (c t) m"))
            c_id = cpool.tile([P, 128], bf16)
            nc.sync.dma_start(c_id[:], id_d)
            c_ones = cpool.tile([P, 128], bf16)
            nc.sync.dma_start(c_ones[:], ones_d)
            c_scl = cpool.tile([P, 2 * S], f32)
            nc.sync.dma_start(c_scl[:].rearrange("p (s m) -> p s m", s=2),
                              scl_d.rearrange("s p m -> p s m"))
            c_bias = cpool.tile([P, 8], f32)
            nc.sync.dma_start(c_bias[:], bias_d)

            keyn8 = kpool.tile([P, 2 * 128], f8)      # [kt, 110(+pad)] key/16
            vT = kpool.tile([S, 512], bf16)           # value^T [s, c]
            qsb8 = kpool.tile([P, 2 * HW], f8)        # q in fp8 [kq, n]

            # ---------------- Phase A ----------------
            with ExitStack() as actx:
                bigp = actx.enter_context(tc.tile_pool(name="bigA", bufs=1))
                xap = actx.enter_context(tc.tile_pool(name="xa", bufs=3))
                blkp = actx.enter_context(tc.tile_pool(name="blk", bufs=2))
                tmpp = actx.enter_context(tc.tile_pool(name="tmpA", bufs=1))

                k1p = bigp.tile([P, HWP2], f8)
                v1p = bigp.tile([P, 2 * HWP2], f8)
                p24 = bigp.tile([P, 6 * 576], f32)
                allp = bigp.tile([P, 6 * S], f32)
                valn = bigp.tile([P, 4 * S], bf16)

                # zero pad borders (rows 0/97, cols 0/97 of each padded map)
                for chv in (k1p[:, 1:1 + HWP], v1p[:, 1:1 + HWP],
                            v1p[:, HWP2 + 1:HWP2 + 1 + HWP]):
                    c3 = chv.rearrange("p (h w) -> p h w", w=WP)
                    nc.gpsimd.memset(c3[:, 0:1, :], 0.0)
                    nc.gpsimd.memset(c3[:, 97:98, :], 0.0)
                    nc.gpsimd.memset(c3[:, 1:97, 0:1], 0.0)
                    nc.gpsimd.memset(c3[:, 1:97, 97:98], 0.0)

                def strip_pool(mapap, base, slot, strip):
                    # 4x4 block sums of 24 rows (6 rowblocks) -> p24
                    off = base + (24 * strip + 1) * WP + 1
                    src = bass.AP(tensor=mapap.tensor, offset=mapap.offset + off,
                                  ap=[mapap.ap[0], [4 * WP, 6], [4, 24],
                                      [WP, 4], [1, 4]])
                    dst = p24[:, slot * 576 + strip * 144:slot * 576 + (strip + 1) * 144]
                    nc.vector.tensor_reduce(
                        dst.rearrange("p (a b) -> p a b", a=6), src,
                        axis=AX.XY, op=ALU.add)

                # primary 1x1 convs + q conv, fp8 DoubleRow, 2 rowblocks per DMA
                with tc.tile_pool(name="psA", bufs=2, space="PSUM") as psA, \
                        tc.tile_pool(name="psQ", bufs=1, space="PSUM") as psQ:
                    for rbb in range(RB // 2):
                        xt = xap.tile([P, 4 * 2 * RBN], f8, name="xt")
                        nc.sync.dma_start(
                            xt[:].rearrange("p (t n) -> p t n", t=4),
                            x8_r[:, :, rbb * 2 * RBN:(rbb + 1) * 2 * RBN])
                        xtv = xt[:].rearrange("p (t n) -> p t n", t=4)
                        wqv = c_wq[:].rearrange("p (t m) -> p t m", t=4)
                        wkv = c_wkp[:].rearrange("p (t m) -> p t m", t=4)
                        wvv = c_wvp[:].rearrange("p (t m) -> p t m", t=4)
                        for sub in range(2):
                            rb = rbb * 2 + sub
                            rhs0 = xtv[:, 0:2, sub * RBN:(sub + 1) * RBN]
                            rhs1 = xtv[:, 2:4, sub * RBN:(sub + 1) * RBN]
                            # maps: k1, v1a, v1b -> ACT relu, fp8 into padded maps
                            dsts = [(k1p, 0, wkv[:, :, 0:128], 0, 0),
                                    (v1p, 0, wvv[:, :, 0:128], 2, 2),
                                    (v1p, 1, wvv[:, :, 128:256], 3, 3)]
                            for di, (dst, half, wt, bcol, slot) in enumerate(dsts):
                                ps = psA.tile([P, RBN], f32, name=f"pps{di}")
                                nc.tensor.matmul(ps[:], wt[:, 0:2, :], rhs0,
                                                 start=True, stop=False, perf_mode=DR)
                                nc.tensor.matmul(ps[:], wt[:, 2:4, :], rhs1,
                                                 start=False, stop=True, perf_mode=DR)
                                dv = dst[:, half * HWP2 + 1:half * HWP2 + 1 + HWP].rearrange(
                                    "p (h w) -> p h w", w=WP)
                                nc.scalar.activation(
                                    dv[:, 4 * rb + 1:4 * rb + 5, 1:97],
                                    ps[:].rearrange("p (h w) -> p h w", w=HH),
                                    AF.Relu, bias=c_bias[:, bcol:bcol + 1])
                            # q halves -> DVE relu, fp8 into qsb8
                            for kq in range(2):
                                qps = psQ.tile([P, RBN], f32, name=f"q{kq}")
                                nc.tensor.matmul(qps[:], wqv[:, 0:2, kq * 128:kq * 128 + 128],
                                                 rhs0, start=True, stop=False, perf_mode=DR)
                                nc.tensor.matmul(qps[:], wqv[:, 2:4, kq * 128:kq * 128 + 128],
                                                 rhs1, start=False, stop=True, perf_mode=DR)
                                nc.vector.tensor_scalar(
                                    qsb8[:, kq * HW + rb * RBN:kq * HW + (rb + 1) * RBN],
                                    qps[:], c_bias[:, 6 + kq:7 + kq], 0.0,
                                    ALU.add, ALU.max)
                        if rbb % 3 == 2:
                            strip = rbb // 3
                            strip_pool(k1p[:], 1, 0, strip)
                            strip_pool(v1p[:], 1, 2, strip)
                            strip_pool(v1p[:], HWP2 + 1, 3, strip)

                # small pools over map range [m0, m1) -> allp columns
                def smallpools(m0, m1):
                    m = m1 - m0
                    allp_v = allp[:, m0 * S:m1 * S].rearrange(
                        "p (m s) -> p m s", s=S)
                    p24s = p24[:, m0 * 576:m1 * 576]
                    nc.vector.reduce_sum(
                        allp_v[:, :, 0:1],
                        p24s.rearrange("p (m s) -> p m s", s=576), axis=AX.X)
                    tmp = tmpp.tile([P, 1152], f32, name="tmp", tag="tmp")
                    nc.vector.reduce_sum(
                        tmp[:, 0:m * 72],
                        p24s.rearrange("p (mh wq ws) -> p mh wq ws", wq=3, ws=8),
                        axis=AX.X)
                    nc.vector.reduce_sum(
                        allp_v[:, :, 1:10],
                        tmp[:, 0:m * 72].rearrange(
                            "p (m hq hs wq) -> p m hq wq hs", m=m, hq=3, hs=8),
                        axis=AX.X)
                    tmp6 = tmpp.tile([P, 1152], f32, name="tmp6", tag="tmp")
                    nc.vector.reduce_sum(
                        tmp6[:, 0:m * 144],
                        p24s.rearrange("p (mh wq ws) -> p mh wq ws", wq=6, ws=4),
                        axis=AX.X)
                    nc.vector.reduce_sum(
                        allp_v[:, :, 10:46],
                        tmp6[:, 0:m * 144].rearrange(
                            "p (m hq hs wq) -> p m hq wq hs", m=m, hq=6, hs=4),
                        axis=AX.X)
                    tmp8 = tmpp.tile([P, 1152], f32, name="tmp8", tag="tmp")
                    nc.vector.reduce_sum(
                        tmp8[:, 0:m * 192],
                        p24s.rearrange("p (mh wq ws) -> p mh wq ws", wq=8, ws=3),
                        axis=AX.X)
                    nc.vector.reduce_sum(
                        allp_v[:, :, 46:110],
                        tmp8[:, 0:m * 192].rearrange(
                            "p (m hq hs wq) -> p m hq wq hs", m=m, hq=8, hs=3),
                        axis=AX.X)

                def vt_build(j):
                    tp = psTp.tile([P, 128], bf16, name="tp", tag="tp")
                    nc.tensor.transpose(tp[0:S, :], valn[:, j * S:(j + 1) * S],
                                        c_id[:])
                    nc.scalar.copy(vT[:, j * 128:(j + 1) * 128], tp[0:S, :])

                def val_finish(m0, m1):
                    smallpools(m0, m1)
                    for mm in range(m0, m1):
                        j = mm - 2
                        nc.vector.tensor_mul(valn[:, j * S:(j + 1) * S],
                                             allp[:, mm * S:(mm + 1) * S],
                                             c_scl[:, S:2 * S])
                        vt_build(j)

                # depthwise 3x3: 2 fp8-DR tap pairs + 5 fp8 singles per rowblock
                with tc.tile_pool(name="psD", bufs=1, space="PSUM") as psD, \
                        tc.tile_pool(name="psTa", bufs=2, space="PSUM") as psTp:
                    val_finish(2, 4)   # v1a, v1b pooled already
                    dgv = c_dg[:].rearrange("p (ct m) -> p ct m", ct=27)
                    chunks = [(0, 1, 1), (1, 4, 4), (2, 5, 5)]
                    for ci, bcol, slot in chunks:
                        chv = (v1p[:, HWP2:2 * HWP2] if ci == 2 else
                               (v1p[:, 0:HWP2] if ci == 1 else k1p[:, 0:HWP2]))
                        m3 = chv[:, 1:1 + HWP].rearrange("p (h w) -> p h w", w=WP)
                        for g in range(4):
                            blk = blkp.tile([P, 6 * RBN], bf16, name="blk")
                            for j in range(6):
                                rb = g * 6 + j
                                dwp = psD.tile([P, 392], f32, name=f"dw{j}")
                                first = True
                                for ta in (2, 5):
                                    dya, dxa = ta // 3, ta % 3
                                    off = (4 * rb + dya) * WP + (dxa - 1) + 1
                                    va = chv[:, off:off + 392]
                                    rhs = bass.AP(
                                        tensor=va.tensor, offset=va.offset,
                                        ap=[va.ap[0], [96, 2], [1, 392]])
                                    nc.tensor.matmul(
                                        dwp[:], dgv[:, ci * 9 + ta:ci * 9 + ta + 2, :],
                                        rhs, start=first, stop=False, perf_mode=DR)
                                    first = False
                                dwv = dwp[:].rearrange(
                                    "p (h w) -> p h w", w=WP)[:, :, 1:97]
                                for t in (0, 1, 4, 7, 8):
                                    dy, dx = t // 3, t % 3
                                    nc.tensor.matmul(
                                        dwv, dgv[:, ci * 9 + t, :],
                                        m3[:, 4 * rb + dy:4 * rb + dy + 4, dx:dx + 96],
                                        start=False, stop=(t == 8))
                                nc.vector.tensor_scalar(
                                    blk[:, j * RBN:(j + 1) * RBN].rearrange(
                                        "p (h w) -> p h w", w=96),
                                    dwv, c_bias[:, bcol:bcol + 1], 0.0,
                                    ALU.add, ALU.max)
                            bv = blk[:].rearrange(
                                "p (r h wq ws) -> p r wq h ws", r=6, h=4, ws=4)
                            nc.vector.tensor_reduce(
                                p24[:, slot * 576 + g * 144:slot * 576 + (g + 1) * 144]
                                .rearrange("p (a b) -> p a b", a=6),
                                bv, axis=AX.XY, op=ALU.add)
                        if ci == 0:
                            smallpools(0, 2)
                            for kq in range(2):
                                nc.vector.tensor_mul(
                                    keyn8[:, kq * 128:kq * 128 + S],
                                    allp[:, kq * S:(kq + 1) * S], c_scl[:, 0:S])
                        elif ci == 1:
                            val_finish(4, 5)
                        else:
                            val_finish(5, 6)

            # ---------------- Phase B: attention / output ----------------
            with ExitStack() as bctx:
                xbp = bctx.enter_context(tc.tile_pool(name="xb", bufs=3))
                pp = bctx.enter_context(tc.tile_pool(name="pexp", bufs=3))
                sp = bctx.enter_context(tc.tile_pool(name="small", bufs=3))
                obp = bctx.enter_context(tc.tile_pool(name="outb", bufs=3))
                psS = bctx.enter_context(tc.tile_pool(name="psS", bufs=2, space="PSUM"))
                psB = bctx.enter_context(tc.tile_pool(name="psB", bufs=2, space="PSUM"))
                psC = bctx.enter_context(tc.tile_pool(name="psC", bufs=2, space="PSUM"))

                knv = keyn8[:].rearrange("p (t m) -> p t m", t=2)[:, :, 0:S]
                qv = qsb8[:].rearrange("p (t m) -> p t m", t=2)

                sims = [None] * NCH
                ets = [None] * NCH
                etns = [None] * NCH

                def do_simT(n):
                    ps = psS.tile([S, NCW], f32, name="sim")
                    nc.tensor.matmul(ps[:], knv,
                                     qv[:, :, n * NCW:(n + 1) * NCW],
                                     start=True, stop=True, perf_mode=DR)
                    sims[n] = ps

                def do_exp(n):
                    eT = pp.tile([S, NCW], bf16, name="eT")
                    nc.scalar.activation(eT[:], sims[n][:], AF.Exp)
                    ets[n] = eT

                def do_sumnorm(n):
                    # sum over s via ones-matmul (broadcast to all partitions)
                    sb = psB.tile([P, NCW], f32, name="sumbc")
                    nc.tensor.matmul(sb[:], c_ones[0:S, :], ets[n][:],
                                     start=True, stop=True)
                    rbc = sp.tile([S, NCW], f32, name="rbc")
                    nc.vector.reciprocal(rbc[:], sb[0:S, :])
                    eTn = pp.tile([S, NCW], bf16, name="eTn")
                    nc.vector.tensor_mul(eTn[:], ets[n][:], rbc[:])
                    etns[n] = eTn

                def do_ctx(n, xbt, outb):
                    for cv in range(4):
                        cps = psC.tile([P, NCW], f32, name="ctx")
                        nc.tensor.matmul(cps[:], vT[:, cv * 128:(cv + 1) * 128],
                                         etns[n][:], start=True,
                                         stop=(cv < 2))
                        if cv < 2:
                            nc.vector.tensor_add(
                                outb[:, cv * NCW:(cv + 1) * NCW], cps[:],
                                xbt[:, cv * NCW:(cv + 1) * NCW])
                        else:
                            nc.tensor.matmul(cps[:], c_id[:],
                                             xbt[:, cv * NCW:(cv + 1) * NCW],
                                             start=False, stop=True)
                            nc.scalar.copy(outb[:, cv * NCW:(cv + 1) * NCW],
                                           cps[:])
                    nc.scalar.dma_start(
                        y_r[:, :, n * NCW:(n + 1) * NCW],
                        outb[:].rearrange("p (t n) -> p t n", t=4))

                xbts = [None] * NCH
                do_simT(0)
                do_exp(0)
                do_sumnorm(0)
                for n in range(NCH):
                    xbt = xbp.tile([P, 4 * NCW], bf16, name="xtb")
                    nc.sync.dma_start(
                        xbt[:].rearrange("p (t n) -> p t n", t=4),
                        xb_r[:, :, n * NCW:(n + 1) * NCW])
                    xbts[n] = xbt
                    if n + 1 < NCH:
                        do_simT(n + 1)
                        do_exp(n + 1)
                    outb = obp.tile([P, 4 * NCW], bf16, name="outb")
                    do_ctx(n, xbts[n][:], outb)
                    if n + 1 < NCH:
                        do_sumnorm(n + 1)

    nc.compile()
    return nc


def prep_host_inputs(inputs):
    """Fold BN affine into weights, quantize to fp8, build aux tensors."""
    import ml_dtypes
    F8 = ml_dtypes.float8_e4m3
    g = lambda a: np.ascontiguousarray(np.asarray(a, dtype=np.float32))
    wq = (g(inputs["q_g"])[:, None] * g(inputs["q_w"])[:, :, 0, 0]).T
    wkp = (g(inputs["kp_g"])[:, None] * g(inputs["kp_w"])[:, :, 0, 0]).T
    wvp = (g(inputs["vp_g"])[:, None] * g(inputs["vp_w"])[:, :, 0, 0]).T
    wkc = g(inputs["kc_g"])[:, None] * g(inputs["kc_w"])[:, 0].reshape(128, 9)
    wvc = g(inputs["vc_g"])[:, None] * g(inputs["vc_w"])[:, 0].reshape(256, 9)

    diag = np.zeros((3, 9, 128, 128), np.float32)
    for t in range(9):
        diag[0, t] = np.diag(wkc[:, t])
        diag[1, t] = np.diag(wvc[:128, t])
        diag[2, t] = np.diag(wvc[128:, t])

    scale110 = np.zeros(S, np.float32)
    scale110[0] = 1.0 / 9216
    scale110[1:10] = 1.0 / 1024
    scale110[10:46] = 1.0 / 256
    scale110[46:110] = 1.0 / 144
    scl = np.zeros((2, 128, S), np.float32)
    scl[0] = scale110 / 16.0
    scl[1] = scale110

    bias = np.zeros((128, 8), np.float32)
    bias[:, 0] = g(inputs["kp_b"])
    bias[:, 1] = g(inputs["kc_b"])
    bias[:, 2] = g(inputs["vp_b"])[:128]
    bias[:, 3] = g(inputs["vp_b"])[128:]
    bias[:, 4] = g(inputs["vc_b"])[:128]
    bias[:, 5] = g(inputs["vc_b"])[128:]
    bias[:, 6] = g(inputs["q_b"])[:128]
    bias[:, 7] = g(inputs["q_b"])[128:]

    return {
        "wq": np.ascontiguousarray(wq).astype(F8),
        "wkp": np.ascontiguousarray(wkp).astype(F8),
        "wvp": np.ascontiguousarray(wvp).astype(F8),
        "diag": diag.astype(F8),
        "ident": np.eye(128, dtype=ml_dtypes.bfloat16),
        "ones": np.ones((128, 128), dtype=ml_dtypes.bfloat16),
        "scl": scl,
        "bias": bias,
    }


def make_in_maps(inputs):
    import ml_dtypes
    host = prep_host_inputs(inputs)
    x = np.asarray(inputs["x"], dtype=np.float32)
    B = x.shape[0]
    in_maps = []
    for b in range(B):
        m = dict(host)
        xf = np.ascontiguousarray(x[b].reshape(512, HW))
        m["x8"] = xf.astype(ml_dtypes.float8_e4m3)
        m["xb"] = xf.astype(ml_dtypes.bfloat16)
        in_maps.append(m)
    return in_maps


_NC = None


def get_nc():
    global _NC
    if _NC is None:
        _NC = build_bass()
    return _NC


def kernel(**inputs):
    from concourse import bass_utils
    nc = get_nc()
    in_maps = make_in_maps(inputs)
    res = bass_utils.run_bass_kernel_spmd(
        nc, in_maps, core_ids=list(range(len(in_maps))), trace=False)
    outs = [r["y"].astype(np.float32).reshape(512, HH, HH) for r in res.results]
    return np.stack(outs, axis=0).astype(np.float32)


# revision 8
# speedup vs baseline: 1.3280x; 1.2521x over previous
"""CAPAttentionModule Trainium2 kernel (v2: fp8 DoubleRow + transposed sim).

Data-parallel over batch: 8 images -> 8 NeuronCores, one image per core.
Per core (x: [512, 9216] = [C, H*W], H=W=96):
  k1 = relu(Wkp x + b)              [128, HW]   fp8 DoubleRow conv (K=512)
  v1 = relu(Wvp x + b)              [256, HW]   fp8 DoubleRow conv
  q  = relu(Wq x + b)               [256, HW]   fp8 DoubleRow conv, kept in fp8
  k2/v2 = relu(dw3x3 + b)           diag matmuls: 2 fp8 DoubleRow tap-pairs
                                    ((2,3),(5,6): delta=96, 16B-aligned) + 5 singles
  key/value = psp pooling           [*, 110]    batched strip reduces on DVE
  simT = key^T q / 16               [110, HW]   ONE fp8 DoubleRow matmul per 512-chunk
  softmax over s (partition dim):   exp on ACT; sum via ones-matmul (broadcast to
                                    all partitions); reciprocal+scale on DVE
  out = x + value @ simT            residual added half on DVE, half via
                                    identity-matmul PSUM accumulation + ACT copy
HBM traffic: read x fp8 (4.7MB) + x bf16 (9.4MB), write y bf16 (9.4MB).
"""

import numpy as np

P = 128
HH = 96
WP = 98          # padded width (zero border ring)
HW = 9216
HWP = WP * WP    # 9604
HWP2 = HWP + 2   # +1 lead pad (for fp8 DR 2B-aligned starts) +1 tail
RB = 24          # row blocks of 4 rows
RBN = 4 * HH     # 384
NCH = 18         # phase-B column chunks
NCW = 512
S = 110


def build_bass():
    import concourse.bacc as bacc
    import concourse.tile as tile
    from concourse import mybir, bass
    from contextlib import ExitStack

    f32 = mybir.dt.float32
    f8 = mybir.dt.float8e4
    bf16 = mybir.dt.bfloat16
    AF = mybir.ActivationFunctionType
    AX = mybir.AxisListType
    ALU = mybir.AluOpType
    DR = mybir.MatmulPerfMode.DoubleRow

    nc = bacc.Bacc("TRN2", target_bir_lowering=False, debug=False,
                   enable_asserts=False, num_devices=8)

    x8_d = nc.dram_tensor("x8", [512, HW], f8, kind="ExternalInput").ap()
    xb_d = nc.dram_tensor("xb", [512, HW], bf16, kind="ExternalInput").ap()
    wq_d = nc.dram_tensor("wq", [512, 256], f8, kind="ExternalInput").ap()
    wkp_d = nc.dram_tensor("wkp", [512, 128], f8, kind="ExternalInput").ap()
    wvp_d = nc.dram_tensor("wvp", [512, 256], f8, kind="ExternalInput").ap()
    diag_d = nc.dram_tensor("diag", [3, 9, 128, 128], f8, kind="ExternalInput").ap()
    id_d = nc.dram_tensor("ident", [128, 128], bf16, kind="ExternalInput").ap()
    ones_d = nc.dram_tensor("ones", [128, 128], bf16, kind="ExternalInput").ap()
    scl_d = nc.dram_tensor("scl", [2, 128, S], f32, kind="ExternalInput").ap()
    bias_d = nc.dram_tensor("bias", [128, 8], f32, kind="ExternalInput").ap()
    y_d = nc.dram_tensor("y", [512, HW], bf16, kind="ExternalOutput").ap()

    x8_r = x8_d.rearrange("(t p) n -> p t n", p=P)
    xb_r = xb_d.rearrange("(t p) n -> p t n", p=P)
    y_r = y_d.rearrange("(t p) n -> p t n", p=P)

    with tile.TileContext(nc) as tc:
        with ExitStack() as top:
            cpool = top.enter_context(tc.tile_pool(name="consts", bufs=1))
            kpool = top.enter_context(tc.tile_pool(name="keep", bufs=1))

            c_wq = cpool.tile([P, 4 * 256], f8)
            nc.sync.dma_start(c_wq[:].rearrange("p (t m) -> p t m", t=4),
                              wq_d.rearrange("(t p) m -> p t m", p=P))
            c_wkp = cpool.tile([P, 4 * 128], f8)
            nc.sync.dma_start(c_wkp[:].rearrange("p (t m) -> p t m", t=4),
                              wkp_d.rearrange("(t p) m -> p t m", p=P))
            c_wvp = cpool.tile([P, 4 * 256], f8)
            nc.sync.dma_start(c_wvp[:].rearrange("p (t m) -> p t m", t=4),
                              wvp_d.rearrange("(t p) m -> p t m", p=P))
            c_dg = cpool.tile([P, 27 * 128], f8)
            nc.sync.dma_start(c_dg[:].rearrange("p (ct m) -> p ct m", ct=27),
                              diag_d.rearrange("c t p m -> p (c t) m"))
            c_id = cpool.tile([P, 128], bf16)
            nc.sync.dma_start(c_id[:], id_d)
            c_ones = cpool.tile([P, 128], bf16)
            nc.sync.dma_start(c_ones[:], ones_d)
            c_scl = cpool.tile([P, 2 * S], f32)
            nc.sync.dma_start(c_scl[:].rearrange("p (s m) -> p s m", s=2),
                              scl_d.rearrange("s p m -> p s m"))
            c_bias = cpool.tile([P, 8], f32)
            nc.sync.dma_start(c_bias[:], bias_d)

            keyn8 = kpool.tile([P, 2 * 128], f8)      # [kt, 110(+pad)] key/16
            vT = kpool.tile([S, 512], bf16)           # value^T [s, c]
            qsb8 = kpool.tile([P, 2 * HW], f8)        # q in fp8 [kq, n]

            # ---------------- Phase A ----------------
            with ExitStack() as actx:
                bigp = actx.enter_context(tc.tile_pool(name="bigA", bufs=1))
                xap = actx.enter_context(tc.tile_pool(name="xa", bufs=3))
                blkp = actx.enter_context(tc.tile_pool(name="blk", bufs=2))
                tmpp = actx.enter_context(tc.tile_pool(name="tmpA", bufs=1))

                k1p = bigp.tile([P, HWP2], f8)
                v1p = bigp.tile([P, 2 * HWP2], f8)
                p24 = bigp.tile([P, 6 * 576], f32)
                allp = bigp.tile([P, 6 * S], f32)
                valn = bigp.tile([P, 4 * S], bf16)

                # zero pad borders (rows 0/97, cols 0/97 of each padded map)
                for chv in (k1p[:, 1:1 + HWP], v1p[:, 1:1 + HWP],
                            v1p[:, HWP2 + 1:HWP2 + 1 + HWP]):
                    c3 = chv.rearrange("p (h w) -> p h w", w=WP)
                    nc.gpsimd.memset(c3[:, 0:1, :], 0.0)
                    nc.gpsimd.memset(c3[:, 97:98, :], 0.0)
                    nc.gpsimd.memset(c3[:, 1:97, 0:1], 0.0)
                    nc.gpsimd.memset(c3[:, 1:97, 97:98], 0.0)

                def strip_pool(mapap, base, slot, strip):
                    # 4x4 block sums of 24 rows (6 rowblocks) -> p24
                    off = base + (24 * strip + 1) * WP + 1
                    src = bass.AP(tensor=mapap.tensor, offset=mapap.offset + off,
                                  ap=[mapap.ap[0], [4 * WP, 6], [4, 24],
                                      [WP, 4], [1, 4]])
                    dst = p24[:, slot * 576 + strip * 144:slot * 576 + (strip + 1) * 144]
                    nc.vector.tensor_reduce(
                        dst.rearrange("p (a b) -> p a b", a=6), src,
                        axis=AX.XY, op=ALU.add)

                # primary 1x1 convs + q conv, fp8 DoubleRow, 2 rowblocks per DMA
                with tc.tile_pool(name="psA", bufs=2, space="PSUM") as psA, \
                        tc.tile_pool(name="psQ", bufs=1, space="PSUM") as psQ:
                    for rbb in range(RB // 2):
                        xt = xap.tile([P, 4 * 2 * RBN], f8, name="xt")
                        nc.sync.dma_start(
                            xt[:].rearrange("p (t n) -> p t n", t=4),
                            x8_r[:, :, rbb * 2 * RBN:(rbb + 1) * 2 * RBN])
                        xtv = xt[:].rearrange("p (t n) -> p t n", t=4)
                        wqv = c_wq[:].rearrange("p (t m) -> p t m", t=4)
                        wkv = c_wkp[:].rearrange("p (t m) -> p t m", t=4)
                        wvv = c_wvp[:].rearrange("p (t m) -> p t m", t=4)
                        for sub in range(2):
                            rb = rbb * 2 + sub
                            rhs0 = xtv[:, 0:2, sub * RBN:(sub + 1) * RBN]
                            rhs1 = xtv[:, 2:4, sub * RBN:(sub + 1) * RBN]
                            # maps: k1, v1a, v1b -> ACT relu, fp8 into padded maps
                            dsts = [(k1p, 0, wkv[:, :, 0:128], 0, 0),
                                    (v1p, 0, wvv[:, :, 0:128], 2, 2),
                                    (v1p, 1, wvv[:, :, 128:256], 3, 3)]
                            for di, (dst, half, wt, bcol, slot) in enumerate(dsts):
                                ps = psA.tile([P, RBN], f32, name=f"pps{di}")
                                nc.tensor.matmul(ps[:], wt[:, 0:2, :], rhs0,
                                                 start=True, stop=False, perf_mode=DR)
                                nc.tensor.matmul(ps[:], wt[:, 2:4, :], rhs1,
                                                 start=False, stop=True, perf_mode=DR)
                                dv = dst[:, half * HWP2 + 1:half * HWP2 + 1 + HWP].rearrange(
                                    "p (h w) -> p h w", w=WP)
                                nc.scalar.activation(
                                    dv[:, 4 * rb + 1:4 * rb + 5, 1:97],
                                    ps[:].rearrange("p (h w) -> p h w", w=HH),
                                    AF.Relu, bias=c_bias[:, bcol:bcol + 1])
                            # q halves -> DVE relu, fp8 into qsb8
                            for kq in range(2):
                                qps = psQ.tile([P, RBN], f32, name=f"q{kq}")
                                nc.tensor.matmul(qps[:], wqv[:, 0:2, kq * 128:kq * 128 + 128],
                                                 rhs0, start=True, stop=False, perf_mode=DR)
                                nc.tensor.matmul(qps[:], wqv[:, 2:4, kq * 128:kq * 128 + 128],
                                                 rhs1, start=False, stop=True, perf_mode=DR)
                                nc.vector.tensor_scalar(
                                    qsb8[:, kq * HW + rb * RBN:kq * HW + (rb + 1) * RBN],
                                    qps[:], c_bias[:, 6 + kq:7 + kq], 0.0,
                                    ALU.add, ALU.max)
                        if rbb % 3 == 2:
                            strip = rbb // 3
                            strip_pool(k1p[:], 1, 0, strip)
                            strip_pool(v1p[:], 1, 2, strip)
                            strip_pool(v1p[:], HWP2 + 1, 3, strip)

                # small pools over map range [m0, m1) -> allp columns
                def smallpools(m0, m1):
                    m = m1 - m0
                    allp_v = allp[:, m0 * S:m1 * S].rearrange(
                        "p (m s) -> p m s", s=S)
                    p24s = p24[:, m0 * 576:m1 * 576]
                    nc.vector.reduce_sum(
                        allp_v[:, :, 0:1],
                        p24s.rearrange("p (m s) -> p m s", s=576), axis=AX.X)
                    tmp = tmpp.tile([P, 1152], f32, name="tmp", tag="tmp")
                    nc.vector.reduce_sum(
                        tmp[:, 0:m * 72],
                        p24s.rearrange("p (mh wq ws) -> p mh wq ws", wq=3, ws=8),
                        axis=AX.X)
                    nc.vector.reduce_sum(
                        allp_v[:, :, 1:10],
                        tmp[:, 0:m * 72].rearrange(
                            "p (m hq hs wq) -> p m hq wq hs", m=m, hq=3, hs=8),
                        axis=AX.X)
                    tmp6 = tmpp.tile([P, 1152], f32, name="tmp6", tag="tmp")
                    nc.vector.reduce_sum(
                        tmp6[:, 0:m * 144],
                        p24s.rearrange("p (mh wq ws) -> p mh wq ws", wq=6, ws=4),
                        axis=AX.X)
                    nc.vector.reduce_sum(
                        allp_v[:, :, 10:46],
                        tmp6[:, 0:m * 144].rearrange(
                            "p (m hq hs wq) -> p m hq wq hs", m=m, hq=6, hs=4),
                        axis=AX.X)
                    tmp8 = tmpp.tile([P, 1152], f32, name="tmp8", tag="tmp")
                    nc.vector.reduce_sum(
                        tmp8[:, 0:m * 192],
                        p24s.rearrange("p (mh wq ws) -> p mh wq ws", wq=8, ws=3),
                        axis=AX.X)
                    nc.vector.reduce_sum(
                        allp_v[:, :, 46:110],
                        tmp8[:, 0:m * 192].rearrange(
                            "p (m hq hs wq) -> p m hq wq hs", m=m, hq=8, hs=3),
                        axis=AX.X)

                def vt_build(j):
                    tp = psTp.tile([P, 128], bf16, name="tp", tag="tp")
                    nc.tensor.transpose(tp[0:S, :], valn[:, j * S:(j + 1) * S],
                                        c_id[:])
                    nc.scalar.copy(vT[:, j * 128:(j + 1) * 128], tp[0:S, :])

                def val_finish(m0, m1):
                    smallpools(m0, m1)
                    for mm in range(m0, m1):
                        j = mm - 2
                        nc.vector.tensor_mul(valn[:, j * S:(j + 1) * S],
                                             allp[:, mm * S:(mm + 1) * S],
                                             c_scl[:, S:2 * S])
                        vt_build(j)

                # depthwise 3x3: 2 fp8-DR tap pairs + 5 fp8 singles per rowblock
                with tc.tile_pool(name="psD", bufs=1, space="PSUM") as psD, \
                        tc.tile_pool(name="psTa", bufs=2, space="PSUM") as psTp:
                    val_finish(2, 4)   # v1a, v1b pooled already
                    dgv = c_dg[:].rearrange("p (ct m) -> p ct m", ct=27)
                    chunks = [(0, 1, 1), (1, 4, 4), (2, 5, 5)]
                    for ci, bcol, slot in chunks:
                        chv = (v1p[:, HWP2:2 * HWP2] if ci == 2 else
                               (v1p[:, 0:HWP2] if ci == 1 else k1p[:, 0:HWP2]))
                        m3 = chv[:, 1:1 + HWP].rearrange("p (h w) -> p h w", w=WP)
                        for g in range(4):
                            blk = blkp.tile([P, 6 * RBN], bf16, name="blk")
                            pss = [psD.tile([P, 392], f32, name=f"dw{j}")
                                   for j in range(6)]
                            # tap-outer order: same stationary 6x consecutive
                            for pi, ta in enumerate((2, 5)):
                                dya, dxa = ta // 3, ta % 3
                                lhsT = dgv[:, ci * 9 + ta:ci * 9 + ta + 2, :]
                                for j in range(6):
                                    rb = g * 6 + j
                                    off = (4 * rb + dya) * WP + (dxa - 1) + 1
                                    va = chv[:, off:off + 392]
                                    rhs = bass.AP(
                                        tensor=va.tensor, offset=va.offset,
                                        ap=[va.ap[0], [96, 2], [1, 392]])
                                    nc.tensor.matmul(
                                        pss[j][:], lhsT, rhs,
                                        start=(pi == 0), stop=False, perf_mode=DR)
                            for t in (0, 1, 4, 7, 8):
                                dy, dx = t // 3, t % 3
                                for j in range(6):
                                    rb = g * 6 + j
                                    dwv = pss[j][:].rearrange(
                                        "p (h w) -> p h w", w=WP)[:, :, 1:97]
                                    nc.tensor.matmul(
                                        dwv, dgv[:, ci * 9 + t, :],
                                        m3[:, 4 * rb + dy:4 * rb + dy + 4, dx:dx + 96],
                                        start=False, stop=(t == 8))
                            for j in range(6):
                                dwv = pss[j][:].rearrange(
                                    "p (h w) -> p h w", w=WP)[:, :, 1:97]
                                nc.scalar.activation(
                                    blk[:, j * RBN:(j + 1) * RBN].rearrange(
                                        "p (h w) -> p h w", w=96),
                                    dwv, AF.Relu, bias=c_bias[:, bcol:bcol + 1])
                            bv = blk[:].rearrange(
                                "p (r h wq ws) -> p r wq h ws", r=6, h=4, ws=4)
                            nc.vector.tensor_reduce(
                                p24[:, slot * 576 + g * 144:slot * 576 + (g + 1) * 144]
                                .rearrange("p (a b) -> p a b", a=6),
                                bv, axis=AX.XY, op=ALU.add)
                        if ci == 0:
                            smallpools(0, 2)
                            for kq in range(2):
                                nc.vector.tensor_mul(
                                    keyn8[:, kq * 128:kq * 128 + S],
                                    allp[:, kq * S:(kq + 1) * S], c_scl[:, 0:S])
                        elif ci == 1:
                            val_finish(4, 5)
                        else:
                            val_finish(5, 6)

            # ---------------- Phase B: attention / output ----------------
            with ExitStack() as bctx:
                xbp = bctx.enter_context(tc.tile_pool(name="xb", bufs=3))
                pp = bctx.enter_context(tc.tile_pool(name="pexp", bufs=3))
                sp = bctx.enter_context(tc.tile_pool(name="small", bufs=3))
                obp = bctx.enter_context(tc.tile_pool(name="outb", bufs=3))
                psS = bctx.enter_context(tc.tile_pool(name="psS", bufs=2, space="PSUM"))
                psB = bctx.enter_context(tc.tile_pool(name="psB", bufs=2, space="PSUM"))
                psC = bctx.enter_context(tc.tile_pool(name="psC", bufs=2, space="PSUM"))

                knv = keyn8[:].rearrange("p (t m) -> p t m", t=2)[:, :, 0:S]
                qv = qsb8[:].rearrange("p (t m) -> p t m", t=2)

                sims = [None] * NCH
                ets = [None] * NCH
                etns = [None] * NCH

                def do_simT(n):
                    ps = psS.tile([S, NCW], f32, name="sim")
                    nc.tensor.matmul(ps[:], knv,
                                     qv[:, :, n * NCW:(n + 1) * NCW],
                                     start=True, stop=True, perf_mode=DR)
                    sims[n] = ps

                def do_exp(n):
                    eT = pp.tile([S, NCW], bf16, name="eT")
                    nc.scalar.activation(eT[:], sims[n][:], AF.Exp)
                    ets[n] = eT

                def do_sumnorm(n):
                    # sum over s via ones-matmul (broadcast to all partitions)
                    sb = psB.tile([P, NCW], f32, name="sumbc")
                    nc.tensor.matmul(sb[:], c_ones[0:S, :], ets[n][:],
                                     start=True, stop=True)
                    rbc = sp.tile([S, NCW], f32, name="rbc")
                    nc.vector.reciprocal_approx_fast(rbc[:], sb[0:S, :])
                    eTn = pp.tile([S, NCW], bf16, name="eTn")
                    nc.vector.tensor_mul(eTn[:], ets[n][:], rbc[:])
                    etns[n] = eTn

                def do_ctx(n, xbt, outb):
                    for cv in range(4):
                        cps = psC.tile([P, NCW], f32, name="ctx")
                        nc.tensor.matmul(cps[:], vT[:, cv * 128:(cv + 1) * 128],
                                         etns[n][:], start=True,
                                         stop=(cv < 2))
                        if cv < 2:
                            nc.vector.tensor_add(
                                outb[:, cv * NCW:(cv + 1) * NCW], cps[:],
                                xbt[:, cv * NCW:(cv + 1) * NCW])
                        else:
                            nc.tensor.matmul(cps[:], c_id[:],
                                             xbt[:, cv * NCW:(cv + 1) * NCW],
                                             start=False, stop=True)
                            nc.scalar.copy(outb[:, cv * NCW:(cv + 1) * NCW],
                                           cps[:])
                    nc.scalar.dma_start(
                        y_r[:, :, n * NCW:(n + 1) * NCW],
                        outb[:].rearrange("p (t n) -> p t n", t=4))

                xbts = [None] * NCH
                do_simT(0)
                do_exp(0)
                do_sumnorm(0)
                for n in range(NCH):
                    xbt = xbp.tile([P, 4 * NCW], bf16, name="xtb")
                    nc.sync.dma_start(
                        xbt[:].rearrange("p (t n) -> p t n", t=4),
                        xb_r[:, :, n * NCW:(n + 1) * NCW])
                    xbts[n] = xbt
                    if n + 1 < NCH:
                        do_simT(n + 1)
                        do_exp(n + 1)
                    outb = obp.tile([P, 4 * NCW], bf16, name="outb")
                    do_ctx(n, xbts[n][:], outb)
                    if n + 1 < NCH:
                        do_sumnorm(n + 1)

    nc.compile()
    return nc


def prep_host_inputs(inputs):
    """Fold BN affine into weights, quantize to fp8, build aux tensors."""
    import ml_dtypes
    F8 = ml_dtypes.float8_e4m3
    g = lambda a: np.ascontiguousarray(np.asarray(a, dtype=np.float32))
    wq = (g(inputs["q_g"])[:, None] * g(inputs["q_w"])[:, :, 0, 0]).T
    wkp = (g(inputs["kp_g"])[:, None] * g(inputs["kp_w"])[:, :, 0, 0]).T
    wvp = (g(inputs["vp_g"])[:, None] * g(inputs["vp_w"])[:, :, 0, 0]).T
    wkc = g(inputs["kc_g"])[:, None] * g(inputs["kc_w"])[:, 0].reshape(128, 9)
    wvc = g(inputs["vc_g"])[:, None] * g(inputs["vc_w"])[:, 0].reshape(256, 9)

    diag = np.zeros((3, 9, 128, 128), np.float32)
    for t in range(9):
        diag[0, t] = np.diag(wkc[:, t])
        diag[1, t] = np.diag(wvc[:128, t])
        diag[2, t] = np.diag(wvc[128:, t])

    scale110 = np.zeros(S, np.float32)
    scale110[0] = 1.0 / 9216
    scale110[1:10] = 1.0 / 1024
    scale110[10:46] = 1.0 / 256
    scale110[46:110] = 1.0 / 144
    scl = np.zeros((2, 128, S), np.float32)
    scl[0] = scale110 / 16.0
    scl[1] = scale110

    bias = np.zeros((128, 8), np.float32)
    bias[:, 0] = g(inputs["kp_b"])
    bias[:, 1] = g(inputs["kc_b"])
    bias[:, 2] = g(inputs["vp_b"])[:128]
    bias[:, 3] = g(inputs["vp_b"])[128:]
    bias[:, 4] = g(inputs["vc_b"])[:128]
    bias[:, 5] = g(inputs["vc_b"])[128:]
    bias[:, 6] = g(inputs["q_b"])[:128]
    bias[:, 7] = g(inputs["q_b"])[128:]

    return {
        "wq": np.ascontiguousarray(wq).astype(F8),
        "wkp": np.ascontiguousarray(wkp).astype(F8),
        "wvp": np.ascontiguousarray(wvp).astype(F8),
        "diag": diag.astype(F8),
        "ident": np.eye(128, dtype=ml_dtypes.bfloat16),
        "ones": np.ones((128, 128), dtype=ml_dtypes.bfloat16),
        "scl": scl,
        "bias": bias,
    }


def make_in_maps(inputs):
    import ml_dtypes
    host = prep_host_inputs(inputs)
    x = np.asarray(inputs["x"], dtype=np.float32)
    B = x.shape[0]
    in_maps = []
    for b in range(B):
        m = dict(host)
        xf = np.ascontiguousarray(x[b].reshape(512, HW))
        m["x8"] = xf.astype(ml_dtypes.float8_e4m3)
        m["xb"] = xf.astype(ml_dtypes.bfloat16)
        in_maps.append(m)
    return in_maps


_NC = None


def get_nc():
    global _NC
    if _NC is None:
        _NC = build_bass()
    return _NC


def kernel(**inputs):
    from concourse import bass_utils
    nc = get_nc()
    in_maps = make_in_maps(inputs)
    res = bass_utils.run_bass_kernel_spmd(
        nc, in_maps, core_ids=list(range(len(in_maps))), trace=False)
    outs = [r["y"].astype(np.float32).reshape(512, HH, HH) for r in res.results]
    return np.stack(outs, axis=0).astype(np.float32)


# revision 17
# speedup vs baseline: 1.3925x; 1.0486x over previous
"""CAPAttentionModule Trainium2 kernel (v2: fp8 DoubleRow + transposed sim).

Data-parallel over batch: 8 images -> 8 NeuronCores, one image per core.
Per core (x: [512, 9216] = [C, H*W], H=W=96):
  k1 = relu(Wkp x + b)              [128, HW]   fp8 DoubleRow conv (K=512)
  v1 = relu(Wvp x + b)              [256, HW]   fp8 DoubleRow conv
  q  = relu(Wq x + b)               [256, HW]   fp8 DoubleRow conv, kept in fp8
  k2/v2 = relu(dw3x3 + b)           diag matmuls: 2 fp8 DoubleRow tap-pairs
                                    ((2,3),(5,6): delta=96, 16B-aligned) + 5 singles
  key/value = psp pooling           [*, 110]    batched strip reduces on DVE
  simT = key^T q / 16               [110, HW]   ONE fp8 DoubleRow matmul per 512-chunk
  softmax over s (partition dim):   exp on ACT; sum via ones-matmul (broadcast to
                                    all partitions); reciprocal+scale on DVE
  out = x + value @ simT            residual added half on DVE, half via
                                    identity-matmul PSUM accumulation + ACT copy
HBM traffic: read x fp8 (4.7MB) + x bf16 (9.4MB), write y bf16 (9.4MB).
"""

import numpy as np

P = 128
HH = 96
WP = 98          # padded width (zero border ring)
HW = 9216
HWP = WP * WP    # 9604
HWP2 = HWP + 2   # +1 lead pad (for fp8 DR 2B-aligned starts) +1 tail
RB = 24          # row blocks of 4 rows
RBN = 4 * HH     # 384
NCH = 18         # phase-B column chunks
NCW = 512
S = 110
CSTR = 9615      # map copy stride (== 15 mod 16 -> aligned fp8 DR kt strides)
CSPAN = 1 + 2 * CSTR + HWP + 10   # one map chunk: 3 copies + lead pad


def build_bass():
    import concourse.bacc as bacc
    import concourse.tile as tile
    from concourse import mybir, bass
    from contextlib import ExitStack

    f32 = mybir.dt.float32
    f8 = mybir.dt.float8e4
    bf16 = mybir.dt.bfloat16
    AF = mybir.ActivationFunctionType
    AX = mybir.AxisListType
    ALU = mybir.AluOpType
    DR = mybir.MatmulPerfMode.DoubleRow

    nc = bacc.Bacc("TRN2", target_bir_lowering=False, debug=False,
                   enable_asserts=False, num_devices=8)

    x8_d = nc.dram_tensor("x8", [512, HW], f8, kind="ExternalInput").ap()
    xb_d = nc.dram_tensor("xb", [512, HW], bf16, kind="ExternalInput").ap()
    wq_d = nc.dram_tensor("wq", [512, 256], f8, kind="ExternalInput").ap()
    wkp_d = nc.dram_tensor("wkp", [512, 128], f8, kind="ExternalInput").ap()
    wvp_d = nc.dram_tensor("wvp", [512, 256], f8, kind="ExternalInput").ap()
    diag_d = nc.dram_tensor("diag", [3, 9, 128, 128], f8, kind="ExternalInput").ap()
    id_d = nc.dram_tensor("ident", [128, 128], bf16, kind="ExternalInput").ap()
    ones_d = nc.dram_tensor("ones", [128, 128], bf16, kind="ExternalInput").ap()
    scl_d = nc.dram_tensor("scl", [2, 128, S], f32, kind="ExternalInput").ap()
    bias_d = nc.dram_tensor("bias", [128, 8], f32, kind="ExternalInput").ap()
    y_d = nc.dram_tensor("y", [512, HW], bf16, kind="ExternalOutput").ap()

    x8_r = x8_d.rearrange("(t p) n -> p t n", p=P)
    xb_r = xb_d.rearrange("(t p) n -> p t n", p=P)
    y_r = y_d.rearrange("(t p) n -> p t n", p=P)

    with tile.TileContext(nc) as tc:
        with ExitStack() as top:
            cpool = top.enter_context(tc.tile_pool(name="consts", bufs=1))
            kpool = top.enter_context(tc.tile_pool(name="keep", bufs=1))

            c_wq = cpool.tile([P, 4 * 256], f8)
            nc.sync.dma_start(c_wq[:].rearrange("p (t m) -> p t m", t=4),
                              wq_d.rearrange("(t p) m -> p t m", p=P))
            c_wkp = cpool.tile([P, 4 * 128], f8)
            nc.sync.dma_start(c_wkp[:].rearrange("p (t m) -> p t m", t=4),
                              wkp_d.rearrange("(t p) m -> p t m", p=P))
            c_wvp = cpool.tile([P, 4 * 256], f8)
            nc.sync.dma_start(c_wvp[:].rearrange("p (t m) -> p t m", t=4),
                              wvp_d.rearrange("(t p) m -> p t m", p=P))
            c_bias = cpool.tile([P, 8], f32)
            nc.sync.dma_start(c_bias[:], bias_d)
            # deferred consts (first needed at dw / phase B; loaded during
            # the primary loop so the first matmul starts sooner)
            c_dg = cpool.tile([P, 27 * 128], f8)
            nc.scalar.dma_start(c_dg[:].rearrange("p (ct m) -> p ct m", ct=27),
                                diag_d.rearrange("c t p m -> p (c t) m"))
            c_id = cpool.tile([P, 128], bf16)
            nc.scalar.dma_start(c_id[:], id_d)
            c_ones = cpool.tile([P, 128], bf16)
            nc.scalar.dma_start(c_ones[:], ones_d)
            c_scl = cpool.tile([P, 2 * S], f32)
            nc.scalar.dma_start(c_scl[:].rearrange("p (s m) -> p s m", s=2),
                                scl_d.rearrange("s p m -> p s m"))

            keyn8 = kpool.tile([P, 2 * 128], f8)      # [kt, 110(+pad)] key/16
            vT = kpool.tile([S, 512], bf16)           # value^T [s, c]
            qsb8 = kpool.tile([P, 2 * HW], f8)        # q in fp8 [kq, n]

            # ---------------- Phase A ----------------
            with ExitStack() as actx:
                bigp = actx.enter_context(tc.tile_pool(name="bigA", bufs=1))
                xap = actx.enter_context(tc.tile_pool(name="xa", bufs=3))
                blkp = actx.enter_context(tc.tile_pool(name="blk", bufs=2))
                tmpp = actx.enter_context(tc.tile_pool(name="tmpA", bufs=1))

                k1p = bigp.tile([P, CSPAN], f8)
                v1p = bigp.tile([P, 2 * CSPAN], f8)
                p24 = bigp.tile([P, 6 * 576], f32)
                allp = bigp.tile([P, 6 * S], f32)
                valn = bigp.tile([P, 4 * S], bf16)

                # zero pad borders (rows 0/97, cols 0/97 of each padded map)
                for chv in (k1p[:, 1:1 + HWP], v1p[:, 1:1 + HWP],
                            v1p[:, CSPAN + 1:CSPAN + 1 + HWP]):
                    c3 = chv.rearrange("p (h w) -> p h w", w=WP)
                    nc.gpsimd.memset(c3[:, 0:1, :], 0.0)
                    nc.gpsimd.memset(c3[:, 97:98, :], 0.0)
                    nc.gpsimd.memset(c3[:, 1:97, 0:1], 0.0)
                    nc.gpsimd.memset(c3[:, 1:97, 97:98], 0.0)

                def strip_pool(mapap, base, slot, strip):
                    # 4x4 block sums of 24 rows (6 rowblocks) -> p24
                    off = base + (24 * strip + 1) * WP + 1
                    src = bass.AP(tensor=mapap.tensor, offset=mapap.offset + off,
                                  ap=[mapap.ap[0], [4 * WP, 6], [4, 24],
                                      [WP, 4], [1, 4]])
                    dst = p24[:, slot * 576 + strip * 144:slot * 576 + (strip + 1) * 144]
                    nc.vector.tensor_reduce(
                        dst.rearrange("p (a b) -> p a b", a=6), src,
                        axis=AX.XY, op=ALU.add)

                def strip_copy(mapap, cb, strip):
                    # replicate newly-written strip rows into copies 2 and 3
                    # (at +CSTR, +2*CSTR; CSTR % 16 == 15 makes the cross-copy
                    # kt strides of fp8 DoubleRow tap-pairs 16B-aligned)
                    r0 = [0, 2450, 4802, 7154][strip]
                    r1 = [2450, 4802, 7154, 9604][strip]
                    src = mapap[:, cb + 1 + r0:cb + 1 + r1]
                    for cc in (1, 2):
                        nc.gpsimd.dma_start(
                            mapap[:, cb + 1 + cc * CSTR + r0:cb + 1 + cc * CSTR + r1],
                            src)

                # primary 1x1 convs + q conv, fp8 DoubleRow, 2 rowblocks per DMA
                with tc.tile_pool(name="psA", bufs=2, space="PSUM") as psA, \
                        tc.tile_pool(name="psQ", bufs=1, space="PSUM") as psQ:
                    for rbb in range(RB // 2):
                        xt = xap.tile([P, 4 * 2 * RBN], f8, name="xt")
                        nc.sync.dma_start(
                            xt[:].rearrange("p (t n) -> p t n", t=4),
                            x8_r[:, :, rbb * 2 * RBN:(rbb + 1) * 2 * RBN])
                        xtv = xt[:].rearrange("p (t n) -> p t n", t=4)
                        wqv = c_wq[:].rearrange("p (t m) -> p t m", t=4)
                        wkv = c_wkp[:].rearrange("p (t m) -> p t m", t=4)
                        wvv = c_wvp[:].rearrange("p (t m) -> p t m", t=4)
                        for sub in range(2):
                            rb = rbb * 2 + sub
                            rhs0 = xtv[:, 0:2, sub * RBN:(sub + 1) * RBN]
                            rhs1 = xtv[:, 2:4, sub * RBN:(sub + 1) * RBN]
                            # maps: k1, v1a, v1b -> ACT relu, fp8 into padded maps
                            dsts = [(k1p, 0, wkv[:, :, 0:128], 0, 0),
                                    (v1p, 0, wvv[:, :, 0:128], 2, 2),
                                    (v1p, 1, wvv[:, :, 128:256], 3, 3)]
                            for di, (dst, half, wt, bcol, slot) in enumerate(dsts):
                                ps = psA.tile([P, RBN], f32, name=f"pps{di}")
                                nc.tensor.matmul(ps[:], wt[:, 0:2, :], rhs0,
                                                 start=True, stop=False, perf_mode=DR)
                                nc.tensor.matmul(ps[:], wt[:, 2:4, :], rhs1,
                                                 start=False, stop=True, perf_mode=DR)
                                dv = dst[:, half * CSPAN + 1:half * CSPAN + 1 + HWP].rearrange(
                                    "p (h w) -> p h w", w=WP)
                                nc.scalar.activation(
                                    dv[:, 4 * rb + 1:4 * rb + 5, 1:97],
                                    ps[:].rearrange("p (h w) -> p h w", w=HH),
                                    AF.Relu, bias=c_bias[:, bcol:bcol + 1])
                            # q halves -> DVE relu, fp8 into qsb8
                            for kq in range(2):
                                qps = psQ.tile([P, RBN], f32, name=f"q{kq}")
                                nc.tensor.matmul(qps[:], wqv[:, 0:2, kq * 128:kq * 128 + 128],
                                                 rhs0, start=True, stop=False, perf_mode=DR)
                                nc.tensor.matmul(qps[:], wqv[:, 2:4, kq * 128:kq * 128 + 128],
                                                 rhs1, start=False, stop=True, perf_mode=DR)
                                nc.vector.tensor_scalar(
                                    qsb8[:, kq * HW + rb * RBN:kq * HW + (rb + 1) * RBN],
                                    qps[:], c_bias[:, 6 + kq:7 + kq], 0.0,
                                    ALU.add, ALU.max)
                        if rbb % 3 == 2:
                            strip = rbb // 3
                            strip_pool(k1p[:], 1, 0, strip)
                            strip_pool(v1p[:], 1, 2, strip)
                            strip_pool(v1p[:], CSPAN + 1, 3, strip)
                            strip_copy(k1p[:], 0, strip)
                            strip_copy(v1p[:], 0, strip)
                            strip_copy(v1p[:], CSPAN, strip)

                # small pools over map range [m0, m1) -> allp columns
                def smallpools(m0, m1):
                    m = m1 - m0
                    allp_v = allp[:, m0 * S:m1 * S].rearrange(
                        "p (m s) -> p m s", s=S)
                    p24s = p24[:, m0 * 576:m1 * 576]
                    nc.vector.reduce_sum(
                        allp_v[:, :, 0:1],
                        p24s.rearrange("p (m s) -> p m s", s=576), axis=AX.X)
                    tmp = tmpp.tile([P, 1152], f32, name="tmp", tag="tmp")
                    nc.vector.reduce_sum(
                        tmp[:, 0:m * 72],
                        p24s.rearrange("p (mh wq ws) -> p mh wq ws", wq=3, ws=8),
                        axis=AX.X)
                    nc.vector.reduce_sum(
                        allp_v[:, :, 1:10],
                        tmp[:, 0:m * 72].rearrange(
                            "p (m hq hs wq) -> p m hq wq hs", m=m, hq=3, hs=8),
                        axis=AX.X)
                    tmp6 = tmpp.tile([P, 1152], f32, name="tmp6", tag="tmp")
                    nc.vector.reduce_sum(
                        tmp6[:, 0:m * 144],
                        p24s.rearrange("p (mh wq ws) -> p mh wq ws", wq=6, ws=4),
                        axis=AX.X)
                    nc.vector.reduce_sum(
                        allp_v[:, :, 10:46],
                        tmp6[:, 0:m * 144].rearrange(
                            "p (m hq hs wq) -> p m hq wq hs", m=m, hq=6, hs=4),
                        axis=AX.X)
                    tmp8 = tmpp.tile([P, 1152], f32, name="tmp8", tag="tmp")
                    nc.vector.reduce_sum(
                        tmp8[:, 0:m * 192],
                        p24s.rearrange("p (mh wq ws) -> p mh wq ws", wq=8, ws=3),
                        axis=AX.X)
                    nc.vector.reduce_sum(
                        allp_v[:, :, 46:110],
                        tmp8[:, 0:m * 192].rearrange(
                            "p (m hq hs wq) -> p m hq wq hs", m=m, hq=8, hs=3),
                        axis=AX.X)

                def vt_build(j):
                    tp = psTp.tile([P, 128], bf16, name="tp", tag="tp")
                    nc.tensor.transpose(tp[0:S, :], valn[:, j * S:(j + 1) * S],
                                        c_id[:])
                    nc.scalar.copy(vT[:, j * 128:(j + 1) * 128], tp[0:S, :])

                def val_finish(m0, m1):
                    smallpools(m0, m1)
                    for mm in range(m0, m1):
                        j = mm - 2
                        nc.vector.tensor_mul(valn[:, j * S:(j + 1) * S],
                                             allp[:, mm * S:(mm + 1) * S],
                                             c_scl[:, S:2 * S])
                        vt_build(j)

                # depthwise 3x3: 2 fp8-DR tap pairs + 5 fp8 singles per rowblock
                with tc.tile_pool(name="psD", bufs=1, space="PSUM") as psD, \
                        tc.tile_pool(name="psTa", bufs=2, space="PSUM") as psTp:
                    val_finish(2, 4)   # v1a, v1b pooled already
                    dgv = c_dg[:].rearrange("p (ct m) -> p ct m", ct=27)
                    chunks = [(0, 1, 1), (1, 4, 4), (2, 5, 5)]
                    # tap pairs for fp8 DoubleRow: (ta, kt0 copy idx, kt delta)
                    pairs = [(0, 0, CSTR + 1), (2, 0, 96),
                             (5, 0, 96), (7, 1, CSTR + 1)]
                    for ci, bcol, slot in chunks:
                        chv = (v1p[:, CSPAN:2 * CSPAN] if ci == 2 else
                               (v1p[:, 0:CSPAN] if ci == 1 else k1p[:, 0:CSPAN]))
                        m3 = chv[:, 1:1 + HWP].rearrange("p (h w) -> p h w", w=WP)
                        for g in range(4):
                            blk = blkp.tile([P, 6 * RBN], bf16, name="blk")
                            pss = [psD.tile([P, 392], f32, name=f"dw{j}")
                                   for j in range(6)]
                            # tap-outer order: same stationary 6x consecutive
                            for pi, (ta, cpi, delta) in enumerate(pairs):
                                dya, dxa = ta // 3, ta % 3
                                lhsT = dgv[:, ci * 9 + ta:ci * 9 + ta + 2, :]
                                for j in range(6):
                                    rb = g * 6 + j
                                    off = (cpi * CSTR + 1
                                           + (4 * rb + dya) * WP + (dxa - 1))
                                    va = chv[:, off:off + 392]
                                    rhs = bass.AP(
                                        tensor=va.tensor, offset=va.offset,
                                        ap=[va.ap[0], [delta, 2], [1, 392]])
                                    nc.tensor.matmul(
                                        pss[j][:], lhsT, rhs,
                                        start=(pi == 0), stop=False, perf_mode=DR)
                            for j in range(6):
                                rb = g * 6 + j
                                dwv = pss[j][:].rearrange(
                                    "p (h w) -> p h w", w=WP)[:, :, 1:97]
                                nc.tensor.matmul(
                                    dwv, dgv[:, ci * 9 + 4, :],
                                    m3[:, 4 * rb + 1:4 * rb + 5, 1:97],
                                    start=False, stop=True)
                            for j in range(6):
                                dwv = pss[j][:].rearrange(
                                    "p (h w) -> p h w", w=WP)[:, :, 1:97]
                                nc.scalar.activation(
                                    blk[:, j * RBN:(j + 1) * RBN].rearrange(
                                        "p (h w) -> p h w", w=96),
                                    dwv, AF.Relu, bias=c_bias[:, bcol:bcol + 1])
                            bv = blk[:].rearrange(
                                "p (r h wq ws) -> p r wq h ws", r=6, h=4, ws=4)
                            nc.vector.tensor_reduce(
                                p24[:, slot * 576 + g * 144:slot * 576 + (g + 1) * 144]
                                .rearrange("p (a b) -> p a b", a=6),
                                bv, axis=AX.XY, op=ALU.add)
                        if ci == 0:
                            smallpools(0, 2)
                            for kq in range(2):
                                nc.vector.tensor_mul(
                                    keyn8[:, kq * 128:kq * 128 + S],
                                    allp[:, kq * S:(kq + 1) * S], c_scl[:, 0:S])
                        elif ci == 1:
                            val_finish(4, 5)
                        else:
                            val_finish(5, 6)

            # ---------------- Phase B: attention / output ----------------
            with ExitStack() as bctx:
                xbp = bctx.enter_context(tc.tile_pool(name="xb", bufs=3))
                pp = bctx.enter_context(tc.tile_pool(name="pexp", bufs=3))
                sp = bctx.enter_context(tc.tile_pool(name="small", bufs=3))
                obp = bctx.enter_context(tc.tile_pool(name="outb", bufs=3))
                psS = bctx.enter_context(tc.tile_pool(name="psS", bufs=2, space="PSUM"))
                psB = bctx.enter_context(tc.tile_pool(name="psB", bufs=2, space="PSUM"))
                psC = bctx.enter_context(tc.tile_pool(name="psC", bufs=2, space="PSUM"))

                knv = keyn8[:].rearrange("p (t m) -> p t m", t=2)[:, :, 0:S]
                qv = qsb8[:].rearrange("p (t m) -> p t m", t=2)

                sims = [None] * NCH
                ets = [None] * NCH
                etns = [None] * NCH

                def do_simT(n):
                    ps = psS.tile([S, NCW], f32, name="sim")
                    nc.tensor.matmul(ps[:], knv,
                                     qv[:, :, n * NCW:(n + 1) * NCW],
                                     start=True, stop=True, perf_mode=DR)
                    sims[n] = ps

                def do_exp(n):
                    eT = pp.tile([S, NCW], bf16, name="eT")
                    nc.scalar.activation(eT[:], sims[n][:], AF.Exp)
                    ets[n] = eT

                def do_sumnorm(n):
                    # sum over s via ones-matmul (broadcast to all partitions)
                    sb = psB.tile([P, NCW], f32, name="sumbc")
                    nc.tensor.matmul(sb[:], c_ones[0:S, :], ets[n][:],
                                     start=True, stop=True)
                    rbc = sp.tile([S, NCW], f32, name="rbc")
                    nc.vector.reciprocal_approx_fast(rbc[:], sb[0:S, :])
                    eTn = pp.tile([S, NCW], bf16, name="eTn")
                    nc.vector.tensor_mul(eTn[:], ets[n][:], rbc[:])
                    etns[n] = eTn

                def do_ctx(n, xbt, outb):
                    for cv in range(4):
                        cps = psC.tile([P, NCW], f32, name="ctx")
                        nc.tensor.matmul(cps[:], vT[:, cv * 128:(cv + 1) * 128],
                                         etns[n][:], start=True,
                                         stop=(cv < 2))
                        if cv < 2:
                            nc.vector.tensor_add(
                                outb[:, cv * NCW:(cv + 1) * NCW], cps[:],
                                xbt[:, cv * NCW:(cv + 1) * NCW])
                        else:
                            nc.tensor.matmul(cps[:], c_id[:],
                                             xbt[:, cv * NCW:(cv + 1) * NCW],
                                             start=False, stop=True)
                            nc.scalar.copy(outb[:, cv * NCW:(cv + 1) * NCW],
                                           cps[:])
                    nc.scalar.dma_start(
                        y_r[:, :, n * NCW:(n + 1) * NCW],
                        outb[:].rearrange("p (t n) -> p t n", t=4))

                xbts = [None] * NCH
                do_simT(0)
                do_exp(0)
                do_sumnorm(0)
                do_simT(1)
                do_exp(1)
                for n in range(NCH):
                    xbt = xbp.tile([P, 4 * NCW], bf16, name="xtb")
                    nc.sync.dma_start(
                        xbt[:].rearrange("p (t n) -> p t n", t=4),
                        xb_r[:, :, n * NCW:(n + 1) * NCW])
                    xbts[n] = xbt
                    if n + 1 < NCH:
                        do_sumnorm(n + 1)
                    if n + 2 < NCH:
                        do_simT(n + 2)
                        do_exp(n + 2)
                    outb = obp.tile([P, 4 * NCW], bf16, name="outb")
                    do_ctx(n, xbts[n][:], outb)

    nc.compile()
    return nc


def prep_host_inputs(inputs):
    """Fold BN affine into weights, quantize to fp8, build aux tensors."""
    import ml_dtypes
    F8 = ml_dtypes.float8_e4m3
    g = lambda a: np.ascontiguousarray(np.asarray(a, dtype=np.float32))
    wq = (g(inputs["q_g"])[:, None] * g(inputs["q_w"])[:, :, 0, 0]).T
    wkp = (g(inputs["kp_g"])[:, None] * g(inputs["kp_w"])[:, :, 0, 0]).T
    wvp = (g(inputs["vp_g"])[:, None] * g(inputs["vp_w"])[:, :, 0, 0]).T
    wkc = g(inputs["kc_g"])[:, None] * g(inputs["kc_w"])[:, 0].reshape(128, 9)
    wvc = g(inputs["vc_g"])[:, None] * g(inputs["vc_w"])[:, 0].reshape(256, 9)

    diag = np.zeros((3, 9, 128, 128), np.float32)
    for t in range(9):
        diag[0, t] = np.diag(wkc[:, t])
        diag[1, t] = np.diag(wvc[:128, t])
        diag[2, t] = np.diag(wvc[128:, t])

    scale110 = np.zeros(S, np.float32)
    scale110[0] = 1.0 / 9216
    scale110[1:10] = 1.0 / 1024
    scale110[10:46] = 1.0 / 256
    scale110[46:110] = 1.0 / 144
    scl = np.zeros((2, 128, S), np.float32)
    scl[0] = scale110 / 16.0
    scl[1] = scale110

    bias = np.zeros((128, 8), np.float32)
    bias[:, 0] = g(inputs["kp_b"])
    bias[:, 1] = g(inputs["kc_b"])
    bias[:, 2] = g(inputs["vp_b"])[:128]
    bias[:, 3] = g(inputs["vp_b"])[128:]
    bias[:, 4] = g(inputs["vc_b"])[:128]
    bias[:, 5] = g(inputs["vc_b"])[128:]
    bias[:, 6] = g(inputs["q_b"])[:128]
    bias[:, 7] = g(inputs["q_b"])[128:]

    return {
        "wq": np.ascontiguousarray(wq).astype(F8),
        "wkp": np.ascontiguousarray(wkp).astype(F8),
        "wvp": np.ascontiguousarray(wvp).astype(F8),
        "diag": diag.astype(F8),
        "ident": np.eye(128, dtype=ml_dtypes.bfloat16),
        "ones": np.ones((128, 128), dtype=ml_dtypes.bfloat16),
        "scl": scl,
        "bias": bias,
    }


def make_in_maps(inputs):
    import ml_dtypes
    host = prep_host_inputs(inputs)
    x = np.asarray(inputs["x"], dtype=np.float32)
    B = x.shape[0]
    in_maps = []
    for b in range(B):
        m = dict(host)
        xf = np.ascontiguousarray(x[b].reshape(512, HW))
        m["x8"] = xf.astype(ml_dtypes.float8_e4m3)
        m["xb"] = xf.astype(ml_dtypes.bfloat16)
        in_maps.append(m)
    return in_maps


_NC = None


def get_nc():
    global _NC
    if _NC is None:
        _NC = build_bass()
    return _NC


def kernel(**inputs):
    from concourse import bass_utils
    nc = get_nc()
    in_maps = make_in_maps(inputs)
    res = bass_utils.run_bass_kernel_spmd(
        nc, in_maps, core_ids=list(range(len(in_maps))), trace=False)
    outs = [r["y"].astype(np.float32).reshape(512, HH, HH) for r in res.results]
    return np.stack(outs, axis=0).astype(np.float32)
